# revision 2
# baseline (speedup 1.0000x reference)
"""Trainium2 kernel for nn_CustomConv1d_6150393168147.

Key algebraic simplification: in the reference, ``on_diag[i, o] =
((o + i) % 256 == o)`` is nonzero only for i == 0 (C_IN == C_OUT == 256),
so

    diag_vals[o] = alpha_topk[0] * V[0, o]
    W[o, c, k]   = diag_vals[o] * (c == o)      for all k in {0, 1, 2}

and the "conv" collapses to a per-channel 3-tap box filter:

    out[n, c, t] = scale[c] * (x[n,c,t-1] + x[n,c,t] + x[n,c,t+1]) + bias[c]

with zero padding at the ends, scale[c] = alpha_topk[0] * V[0, c].

The Dykstra top-k projection only couples channels through a scalar sum,
is O(C * n_iter) and is computed on the host (float32, faithful to the
reference op-for-op).  The streaming part runs on 8 NeuronCores,
data-parallel over batch (1 batch element per core).

Device-side design (per core, per tile of w time-samples x 128 channels):
  - x and out cross HBM as float16 (host converts; rel-err budget 2e-2,
    fp16 quantization contributes ~1e-3), halving DMA bytes.
  - loads/stores are spread across all three DMA-issue queues (SP and ACT
    HWDGE rings + Pool SWDGE) so the streams overlap.
  - add1 (x[t-1] + x[t+1]): DVE tensor_add (2x fp16 mode), or DVE
    tensor_copy (4x) + Pool accum-DMA.
  - add2 (+ x[t]): DVE tensor_add, Pool tensor_add, or an SBUF->SBUF
    accumulating DMA on the Pool SWDGE stream.
  - scale/bias: ACT activation (Identity with per-partition scale/bias
    APs) or DVE tensor_scalar (4x fp16 mode).
The per-tile engine assignment vectors below were tuned against the
CoreSim cost model.
"""

import os
import sys

import numpy as np

for _p in ("/opt/trn_rl_repo", "/root/.axon_site/_ro/trn_rl_repo"):
    if os.path.isdir(_p) and _p not in sys.path:
        sys.path.insert(0, _p)

import concourse.bacc as bacc
import concourse.mybir as mybir
from concourse.bass_utils import run_bass_kernel_spmd
from concourse.tile import TileContext

# Problem constants (hardcoded per the harness contract).
B, C, L = 8, 256, 16384
NCORES = 8
PBLK = C // 128  # partition blocks per core
K_TOP, ALPHA_LR, N_ITER = 16, 0.01, 50

# Tuned schedule (CoreSim cost model).  One code per tile:
#   ld/st: s=SP  a=ACT  p=Pool(SWDGE)      queues
#   add1:  v=DVE tensor_add  c=DVE copy + Pool accum-DMA
#   add2:  v=DVE  p=Pool tensor_add  d=Pool accum-DMA
#   sb:    a=ACT activation  v=DVE tensor_scalar
CONFIG = dict(
    tfree=2048,
    xbufs=6,
    ybufs=6,
    ld="s",
    st="ap|ap|aps",
    add1="v",
    add2="dvp",
    sb="av",
    cq="a",
    warm_act=False,
    interleave=True,
)


def _alpha_topk0(alpha: np.ndarray) -> np.float32:
    """Dykstra sparse-soft-topk projection (float32, mirrors reference);
    returns element 0 of the projected vector, the only one used."""
    f32 = np.float32
    y = alpha.astype(np.float32) / f32(ALPHA_LR)
    p = np.zeros_like(y)
    q = np.zeros_like(y)
    n = f32(y.shape[0])
    k = f32(K_TOP)
    for _ in range(N_ITER):
        u = y + p
        z = u - (np.sum(u, dtype=np.float32) - k) / n
        p = u - z
        v = z + q
        y = np.clip(v, f32(0.0), f32(1.0))
        q = v - y
    return y[0]


def _expand(pat, n):
    if "|" in pat:
        phases = pat.split("|")
        k = len(phases)
        return "".join(
            phases[min(i * k // n, k - 1)][i % len(phases[min(i * k // n, k - 1)])]
            for i in range(n)
        )
    return "".join(pat[i % len(pat)] for i in range(n))


_NC_CACHE = {}


def _build(cfg=None):
    cfg = dict(CONFIG if cfg is None else cfg)
    key = repr(sorted(cfg.items()))
    if key in _NC_CACHE:
        return _NC_CACHE[key]

    tfree = cfg["tfree"]
    f32 = mybir.dt.float32
    f16 = mybir.dt.float16
    # Bacc (not plain Bass): its finalize() runs generate_event_semaphores(),
    # which legalizes the TRN2 1-sync-wait-per-instruction cap.
    nc = bacc.Bacc(None, target_bir_lowering=False, debug=False, num_devices=NCORES)
    xd = nc.declare_dram_parameter("x", [PBLK, 128, L], f16, isOutput=False)
    sd = nc.declare_dram_parameter("scale", [PBLK, 128, 1], f32, isOutput=False)
    bd = nc.declare_dram_parameter("bias", [PBLK, 128, 1], f32, isOutput=False)
    od = nc.declare_dram_parameter("out", [PBLK, 128, L], f16, isOutput=True)

    def eng(code):
        return {"s": nc.sync, "a": nc.scalar, "p": nc.gpsimd, "v": nc.vector}[code]

    nt = L // tfree
    ntot = PBLK * nt
    lds = _expand(cfg["ld"], ntot)
    sts = _expand(cfg["st"], ntot)
    add1s = _expand(cfg["add1"], ntot)
    add2s = _expand(cfg["add2"], ntot)
    sbs = _expand(cfg["sb"], ntot)

    with TileContext(nc) as tc:
        with (
            tc.tile_pool(name="const", bufs=1) as cpool,
            tc.tile_pool(name="xin", bufs=cfg["xbufs"]) as xpool,
            tc.tile_pool(name="yout", bufs=cfg["ybufs"]) as ypool,
        ):
            consts = []
            for b in range(PBLK):
                sct = cpool.tile([128, 1], f32, tag=f"sc{b}")
                bit = cpool.tile([128, 1], f32, tag=f"bi{b}")
                eng(cfg["cq"]).dma_start(out=sct[:], in_=sd[b])
                eng(cfg["cq"]).dma_start(out=bit[:], in_=bd[b])
                consts.append((sct, bit))
            if cfg["warm_act"]:
                wt = cpool.tile([128, 1], f32, tag="warm")
                nc.vector.memset(wt[:], 0.0)
                nc.scalar.activation(
                    out=wt[:], in_=wt[:],
                    func=mybir.ActivationFunctionType.Identity,
                    bias=consts[0][1][:, 0:1], scale=consts[0][0][:, 0:1],
                )

            tiles = [(b, j * tfree, tfree) for b in range(PBLK) for j in range(nt)]
            if cfg["interleave"]:
                tiles = [tiles[i // 2 + (i % 2) * nt] for i in range(ntot)]

            for i, (b, t0, w) in enumerate(tiles):
                # ---- load (with 1-sample halo, zero padding at ends) ----
                q = eng(lds[i])
                xt = xpool.tile([128, w + 2], f16, tag="x")
                if t0 == 0:
                    nc.vector.memset(xt[:, 0:1], 0.0)
                    q.dma_start(out=xt[:, 1 : w + 2], in_=xd[b, :, 0 : w + 1])
                elif t0 + w == L:
                    nc.vector.memset(xt[:, w + 1 : w + 2], 0.0)
                    q.dma_start(out=xt[:, 0 : w + 1], in_=xd[b, :, t0 - 1 : L])
                else:
                    q.dma_start(out=xt[:], in_=xd[b, :, t0 - 1 : t0 + w + 1])

                # ---- compute ----
                sct, bit = consts[b]
                yt = ypool.tile([128, w], f16, tag="y")
                if add1s[i] == "v":
                    nc.vector.tensor_add(
                        out=yt[:], in0=xt[:, 0:w], in1=xt[:, 2 : w + 2]
                    )
                else:  # "c"
                    nc.vector.tensor_copy(out=yt[:], in_=xt[:, 0:w])
                    nc.gpsimd.dma_start(
                        out=yt[:], in_=xt[:, 2 : w + 2],
                        accum_op=mybir.AluOpType.add,
                    )
                a2 = add2s[i]
                if a2 == "d":
                    nc.gpsimd.dma_start(
                        out=yt[:], in_=xt[:, 1 : w + 1],
                        accum_op=mybir.AluOpType.add,
                    )
                else:
                    eng(a2).tensor_add(out=yt[:], in0=yt[:], in1=xt[:, 1 : w + 1])
                if sbs[i] == "a":
                    nc.scalar.activation(
                        out=yt[:], in_=yt[:],
                        func=mybir.ActivationFunctionType.Identity,
                        bias=bit[:, 0:1], scale=sct[:, 0:1],
                    )
                else:
                    nc.vector.tensor_scalar(
                        out=yt[:], in0=yt[:],
                        scalar1=sct[:, 0:1], scalar2=bit[:, 0:1],
                        op0=mybir.AluOpType.mult, op1=mybir.AluOpType.add,
                    )

                # ---- store ----
                eng(sts[i]).dma_start(out=od[b, :, t0 : t0 + w], in_=yt[:])

    nc.finalize()
    _NC_CACHE[key] = nc
    return nc


def run(x, V, alpha, bias, **spmd_kwargs):
    """Returns (out [B,C,L] f32, BassKernelResults)."""
    x = np.asarray(x, dtype=np.float32)
    V = np.asarray(V, dtype=np.float32)
    alpha = np.asarray(alpha, dtype=np.float32)
    bias = np.asarray(bias, dtype=np.float32)

    a0 = _alpha_topk0(alpha)
    scale = (a0 * V[0, :]).astype(np.float32)  # [C]

    nc = _build()
    xs = np.ascontiguousarray(
        x.reshape(B, PBLK, 128, L).astype(np.float16)
    )
    sd = np.ascontiguousarray(scale.reshape(PBLK, 128, 1))
    bd = np.ascontiguousarray(bias.reshape(PBLK, 128, 1))
    in_maps = [{"x": xs[i], "scale": sd, "bias": bd} for i in range(NCORES)]
    res = run_bass_kernel_spmd(nc, in_maps, core_ids=list(range(NCORES)), **spmd_kwargs)
    out = np.stack(
        [
            np.asarray(res.results[i]["out"]).reshape(C, L).astype(np.float32)
            for i in range(NCORES)
        ],
        axis=0,
    )
    return out, res


def kernel(x, V, alpha, bias):
    out, _ = run(x, V, alpha, bias)
    return out


# revision 8
# speedup vs baseline: 1.2035x; 1.2035x over previous
"""Trainium2 kernel for nn_CustomConv1d_6150393168147.

Key algebraic simplification: in the reference, ``on_diag[i, o] =
((o + i) % 256 == o)`` is nonzero only for i == 0 (C_IN == C_OUT == 256),
so

    diag_vals[o] = alpha_topk[0] * V[0, o]
    W[o, c, k]   = diag_vals[o] * (c == o)      for all k in {0, 1, 2}

and the "conv" collapses to a per-channel 3-tap box filter:

    out[n, c, t] = scale[c] * (x[n,c,t-1] + x[n,c,t] + x[n,c,t+1]) + bias[c]

with zero padding at the ends, scale[c] = alpha_topk[0] * V[0, c].

The Dykstra top-k projection only couples channels through a scalar sum,
is O(C * n_iter) and is computed on the host (float32, faithful to the
reference op-for-op).  The streaming part runs on 8 NeuronCores,
data-parallel over batch (1 batch element per core).

Device-side design (per core, per tile of w time-samples x 128 channels):
  - x and out cross HBM as float16 (host converts; rel-err budget 2e-2,
    fp16 quantization contributes ~1e-3), halving DMA bytes.
  - loads/stores are spread across all three DMA-issue queues (SP and ACT
    HWDGE rings + Pool SWDGE) so the streams overlap.
  - add1 (x[t-1] + x[t+1]): DVE tensor_add (2x fp16 mode), or DVE
    tensor_copy (4x) + Pool accum-DMA.
  - add2 (+ x[t]): DVE tensor_add, Pool tensor_add, or an SBUF->SBUF
    accumulating DMA on the Pool SWDGE stream.
  - scale/bias: ACT activation (Identity with per-partition scale/bias
    APs) or DVE tensor_scalar (4x fp16 mode).
The per-tile engine assignment vectors below were tuned against the
CoreSim cost model.
"""

import os
import sys

import numpy as np

for _p in ("/opt/trn_rl_repo", "/root/.axon_site/_ro/trn_rl_repo"):
    if os.path.isdir(_p) and _p not in sys.path:
        sys.path.insert(0, _p)

import concourse.bacc as bacc
import concourse.mybir as mybir
from concourse.bass_utils import run_bass_kernel_spmd
from concourse.tile import TileContext

# Problem constants (hardcoded per the harness contract).
B, C, L = 8, 256, 16384
NCORES = 8
PBLK = C // 128  # partition blocks per core
K_TOP, ALPHA_LR, N_ITER = 16, 0.01, 50

# Tuned schedule (CoreSim cost model).  One code per tile:
#   ld/st: s=SP  a=ACT  p=Pool(SWDGE)      queues
#   add1:  v=DVE tensor_add  c=DVE copy + Pool accum-DMA
#   add2:  v=DVE  p=Pool tensor_add  d=Pool accum-DMA
#   sb:    a=ACT activation  v=DVE tensor_scalar
CONFIG = dict(
    tfree=2048,
    ramp=(1024,),
    xbufs=8,
    ybufs=8,
    ld="saassasssaasaassss",
    st="assssssasaassapaps",
    add1="vvvvvvvvvvvvvvvvvv",
    add2="ppppppppppppppvpvv",
    sb="avavavvvavvvaavvvv",
    cq="p",
    warm_act=True,
    interleave=True,
)


def _alpha_topk0(alpha: np.ndarray) -> np.float32:
    """Dykstra sparse-soft-topk projection (float32, mirrors reference);
    returns element 0 of the projected vector, the only one used."""
    f32 = np.float32
    y = alpha.astype(np.float32) / f32(ALPHA_LR)
    p = np.zeros_like(y)
    q = np.zeros_like(y)
    n = f32(y.shape[0])
    k = f32(K_TOP)
    for _ in range(N_ITER):
        u = y + p
        z = u - (np.sum(u, dtype=np.float32) - k) / n
        p = u - z
        v = z + q
        y = np.clip(v, f32(0.0), f32(1.0))
        q = v - y
    return y[0]


def _widths(tfree, ramp=()):
    mid = L - 2 * sum(ramp)
    k, rem = divmod(mid, tfree)
    ws = list(ramp) + [tfree] * k + ([rem] if rem else []) + list(ramp[::-1])
    assert sum(ws) == L and all(w >= 256 for w in ws), ws
    return ws


def _expand(pat, n):
    if "|" in pat:
        phases = pat.split("|")
        k = len(phases)
        return "".join(
            phases[min(i * k // n, k - 1)][i % len(phases[min(i * k // n, k - 1)])]
            for i in range(n)
        )
    return "".join(pat[i % len(pat)] for i in range(n))


_NC_CACHE = {}


def _build(cfg=None):
    cfg = dict(CONFIG if cfg is None else cfg)
    key = repr(sorted(cfg.items()))
    if key in _NC_CACHE:
        return _NC_CACHE[key]

    tfree = cfg["tfree"]
    f32 = mybir.dt.float32
    f16 = mybir.dt.float16
    # Bacc (not plain Bass): its finalize() runs generate_event_semaphores(),
    # which legalizes the TRN2 1-sync-wait-per-instruction cap.
    nc = bacc.Bacc(None, target_bir_lowering=False, debug=False, num_devices=NCORES)
    xd = nc.declare_dram_parameter("x", [PBLK, 128, L], f16, isOutput=False)
    sd = nc.declare_dram_parameter("scale", [PBLK, 128, 1], f32, isOutput=False)
    bd = nc.declare_dram_parameter("bias", [PBLK, 128, 1], f32, isOutput=False)
    od = nc.declare_dram_parameter("out", [PBLK, 128, L], f16, isOutput=True)

    def eng(code):
        return {"s": nc.sync, "a": nc.scalar, "p": nc.gpsimd, "v": nc.vector}[code]

    ws = _widths(tfree, tuple(cfg.get("ramp", ())))
    nt = len(ws)
    ntot = PBLK * nt
    lds = _expand(cfg["ld"], ntot)
    sts = _expand(cfg["st"], ntot)
    add1s = _expand(cfg["add1"], ntot)
    add2s = _expand(cfg["add2"], ntot)
    sbs = _expand(cfg["sb"], ntot)

    with TileContext(nc) as tc:
        with (
            tc.tile_pool(name="const", bufs=1) as cpool,
            tc.tile_pool(name="xin", bufs=cfg["xbufs"]) as xpool,
            tc.tile_pool(name="yout", bufs=cfg["ybufs"]) as ypool,
        ):
            consts = []
            for b in range(PBLK):
                sct = cpool.tile([128, 1], f32, tag=f"sc{b}")
                bit = cpool.tile([128, 1], f32, tag=f"bi{b}")
                eng(cfg["cq"]).dma_start(out=sct[:], in_=sd[b])
                eng(cfg["cq"]).dma_start(out=bit[:], in_=bd[b])
                consts.append((sct, bit))
            if cfg["warm_act"]:
                wt = cpool.tile([128, 1], f32, tag="warm")
                nc.vector.memset(wt[:], 0.0)
                nc.scalar.activation(
                    out=wt[:], in_=wt[:],
                    func=mybir.ActivationFunctionType.Identity,
                    bias=consts[0][1][:, 0:1], scale=consts[0][0][:, 0:1],
                )

            offs = []
            t0 = 0
            for w in ws:
                offs.append(t0)
                t0 += w
            tiles = [(b, offs[j], ws[j]) for b in range(PBLK) for j in range(nt)]
            if cfg["interleave"]:
                tiles = [tiles[i // 2 + (i % 2) * nt] for i in range(ntot)]

            for i, (b, t0, w) in enumerate(tiles):
                # ---- load (with 1-sample halo, zero padding at ends) ----
                q = eng(lds[i])
                xt = xpool.tile([128, w + 2], f16, tag="x")
                if t0 == 0:
                    nc.vector.memset(xt[:, 0:1], 0.0)
                    q.dma_start(out=xt[:, 1 : w + 2], in_=xd[b, :, 0 : w + 1])
                elif t0 + w == L:
                    nc.vector.memset(xt[:, w + 1 : w + 2], 0.0)
                    q.dma_start(out=xt[:, 0 : w + 1], in_=xd[b, :, t0 - 1 : L])
                else:
                    q.dma_start(out=xt[:], in_=xd[b, :, t0 - 1 : t0 + w + 1])

                # ---- compute ----
                sct, bit = consts[b]
                yt = ypool.tile([128, w], f16, tag="y")
                if add1s[i] == "v":
                    nc.vector.tensor_add(
                        out=yt[:], in0=xt[:, 0:w], in1=xt[:, 2 : w + 2]
                    )
                else:  # "c"
                    nc.vector.tensor_copy(out=yt[:], in_=xt[:, 0:w])
                    nc.gpsimd.dma_start(
                        out=yt[:], in_=xt[:, 2 : w + 2],
                        accum_op=mybir.AluOpType.add,
                    )
                a2 = add2s[i]
                if a2 == "d":
                    nc.gpsimd.dma_start(
                        out=yt[:], in_=xt[:, 1 : w + 1],
                        accum_op=mybir.AluOpType.add,
                    )
                else:
                    eng(a2).tensor_add(out=yt[:], in0=yt[:], in1=xt[:, 1 : w + 1])
                if sbs[i] == "a":
                    nc.scalar.activation(
                        out=yt[:], in_=yt[:],
                        func=mybir.ActivationFunctionType.Identity,
                        bias=bit[:, 0:1], scale=sct[:, 0:1],
                    )
                else:
                    nc.vector.tensor_scalar(
                        out=yt[:], in0=yt[:],
                        scalar1=sct[:, 0:1], scalar2=bit[:, 0:1],
                        op0=mybir.AluOpType.mult, op1=mybir.AluOpType.add,
                    )

                # ---- store ----
                eng(sts[i]).dma_start(out=od[b, :, t0 : t0 + w], in_=yt[:])

    nc.finalize()
    _NC_CACHE[key] = nc
    return nc


def run(x, V, alpha, bias, **spmd_kwargs):
    """Returns (out [B,C,L] f32, BassKernelResults)."""
    x = np.asarray(x, dtype=np.float32)
    V = np.asarray(V, dtype=np.float32)
    alpha = np.asarray(alpha, dtype=np.float32)
    bias = np.asarray(bias, dtype=np.float32)

    a0 = _alpha_topk0(alpha)
    scale = (a0 * V[0, :]).astype(np.float32)  # [C]

    nc = _build()
    xs = np.ascontiguousarray(
        x.reshape(B, PBLK, 128, L).astype(np.float16)
    )
    sd = np.ascontiguousarray(scale.reshape(PBLK, 128, 1))
    bd = np.ascontiguousarray(bias.reshape(PBLK, 128, 1))
    in_maps = [{"x": xs[i], "scale": sd, "bias": bd} for i in range(NCORES)]
    res = run_bass_kernel_spmd(nc, in_maps, core_ids=list(range(NCORES)), **spmd_kwargs)
    out = np.stack(
        [
            np.asarray(res.results[i]["out"]).reshape(C, L).astype(np.float32)
            for i in range(NCORES)
        ],
        axis=0,
    )
    return out, res


def kernel(x, V, alpha, bias):
    out, _ = run(x, V, alpha, bias)
    return out
